# revision 5
# baseline (speedup 1.0000x reference)
"""Trainium2 Bass kernel for nn_MultiHeadAttention_88210038326473 (v2, fp8).

Reference computation (B=4, S=2048, HID=2048, H=16, DH=128):
    Q = queries @ Wq.T + bq ; K = keys @ Wk.T + bk ; V = keys @ Wv.T + bv
    per-head scores = Qh Kh^T / sqrt(HID), key-padding + causal mask,
    softmax, out = attn @ Vh, concat heads, + queries residual.

Sharding: 8 cores = 4 batches x 2 head-groups (8 heads each). Each core
computes out[b, :, hg*1024:(hg+1)*1024] (stored transposed [1024, 2048]
bf16; host transposes back and assembles).

v2 design (all SBUF-resident, fp8 DoubleRow matmuls, fused pipeline):
  - Projections contract HID=2048 as 8 fp8-DoubleRow matmuls of K=256
    (pair layout hid = f*256 + j*128 + p). Inputs kT8/qT8 and weights
    (et-major) are pre-quantized e4m3 on host. fp8 inputs/weights +
    fp8 Q/K for scores measure 4e-3 rel err vs the 2e-2 budget.
  - K and Q projections write fp8 pair-layout tiles kt8/qt8 [64,(2,S)]
    per head (d = j*64 + p), so scores are fp8 DoubleRow too.
  - V projection writes bf16 vfull [128,(st,e)] (partition = s%128).
  - Scores: sT[k,q] DoubleRow per 128-k-tile into PSUM fp32; exp on the
    scalar engine (no max subtraction: scores are O(1)); probabilities
    stay bf16 (fp8 probs would breach the 2e-2 budget); causal diagonal
    via a 0/1 triangle multiply on DVE; attn@V in bf16; row sums via a
    ones-column matmul per k-tile; reciprocal + ones-row broadcast
    matmul; normalize on DVE + bf16 residual added on the Pool engine.
  - Chunk-level pipeline: for each 512-row s-chunk, project K(sc),
    Q(sc), V(sc) then run attention for q-chunk sc over all heads, so
    chunk sc+1's projections overlap attention sc across engines.
    Projection psum drains alternate Activation/DVE ("mix"); the last
    chunk's drains go to DVE to keep Activation free for the exp-heavy
    tail. TimelineSim: 309us vs 624us for the bf16 phased baseline.
"""

import math

import numpy as np

B, S, HID, H, DH = 4, 2048, 2048, 16, 128
NCORES = 8
HPC = 8          # heads per core
EH = HPC * DH    # 1024 e-dims per core
SCALE = 1.0 / math.sqrt(HID)
QC = 512         # attention q-chunk
NQC = S // QC    # 4
NKT = S // DH    # 16 k-tiles
NDR = HID // 256  # 8 DoubleRow contraction tiles
PC = 512         # projection s-chunk
NPC = S // PC    # 4
NEG_BIAS = np.float32(-1.0e30)
COMPUTE_MAX_WAITS = 1


CTRL_OPS = ("InstDrain", "InstNoOp", "InstEventSemaphore", "InstISA")


def _split_excess_waits(nc, max_waits=1, compute_max_waits=None):
    """walrus in this container rejects >1 sem-wait per CTRL-class instruction.
    Move excess waits onto preceding NoOps on the same engine."""
    import concourse.mybir as mybir

    if compute_max_waits is None:
        compute_max_waits = max_waits
    n_split = 0
    for fn in nc.m.functions:
        for blk in fn.blocks:
            insts = list(blk.instructions)
            out = []
            changed = False
            for ins in insts:
                lim = (
                    max_waits
                    if type(ins).__name__ in CTRL_OPS
                    else compute_max_waits
                )
                si = ins.sync_info
                if si is not None and si.on_wait and len(si.on_wait) > lim:
                    waits = list(si.on_wait)
                    carriers, rest = waits[:-lim], waits[-lim:]
                    for i in range(0, len(carriers), max_waits):
                        chunk = carriers[i : i + max_waits]
                        out.append(
                            mybir.InstNoOp(
                                name=f"{ins.name}-ws{i}",
                                engine=ins.engine,
                                bass_nofuse=True,
                                sync_info=mybir.SyncInfo(on_wait=chunk, on_update=[]),
                            )
                        )
                        n_split += 1
                    ins.sync_info = mybir.SyncInfo(
                        on_wait=rest, on_update=list(si.on_update)
                    )
                    changed = True
                out.append(ins)
            if changed:
                blk.instructions = out
    return n_split


_CACHE = {}


def _build(fast=True, phases=("kv", "q", "attn"), reps=1, scale=None, opts=None):
    """Build the (core-uniform) Bass program. Returns nc."""
    scale = scale or {}
    opts = dict(opts or {})
    opts.setdefault("drains", "mix")   # "act" | "dve" | "mix"
    opts.setdefault("rowsum", "pe")    # "pe" | "pool"
    opts.setdefault("bcast", "pe")     # "pe" | "pool"
    opts.setdefault("last_dve", True)  # route last-chunk drains to DVE
    opts.setdefault("pp", 2)
    opts.setdefault("pss", 3)
    opts.setdefault("radd", "pool")
    opts.setdefault("qv_order", "qv")
    opts.setdefault("fused", True)     # chunk-level KV+Q+attn pipeline
    key = ("nc2", fast, tuple(phases), reps, tuple(sorted(scale.items())),
           tuple(sorted(opts.items())))
    if key in _CACHE:
        return _CACHE[key]

    import concourse.bass as bass
    import concourse.mybir as mybir
    from concourse.tile import TileContext

    F32 = mybir.dt.float32
    F32R = mybir.dt.float32r
    F8 = mybir.dt.float8e4
    BF16 = mybir.dt.bfloat16
    EXP = mybir.ActivationFunctionType.Exp
    IDENT = mybir.ActivationFunctionType.Identity

    nc = bass.Bass("TRN2", target_bir_lowering=False, debug=False)

    # inputs (pair layout hid = f*256 + j*128 + p; s-chunk-major for
    # contiguous chunk DMA)
    qT8 = nc.dram_tensor("qT8", [DH, NPC * NDR * 2 * PC], F8, kind="ExternalInput")
    kT8 = nc.dram_tensor("kT8", [DH, NPC * NDR * 2 * PC], F8, kind="ExternalInput")
    wq8 = nc.dram_tensor("wq8", [DH, NDR * 2 * EH], F8, kind="ExternalInput")
    wk8 = nc.dram_tensor("wk8", [DH, NDR * 2 * EH], F8, kind="ExternalInput")
    wv8 = nc.dram_tensor("wv8", [DH, NDR * 2 * EH], F8, kind="ExternalInput")
    bq_d = nc.dram_tensor("bq_d", [DH, HPC], F32, kind="ExternalInput")
    bk_d = nc.dram_tensor("bk_d", [DH, HPC], F32, kind="ExternalInput")
    bv_d = nc.dram_tensor("bv_d", [1, EH], BF16, kind="ExternalInput")
    kbias_d = nc.dram_tensor("kbias_d", [DH, NKT], F32, kind="ExternalInput")
    tri_d = nc.dram_tensor("tri_d", [DH, DH], BF16, kind="ExternalInput")
    ones_c_d = nc.dram_tensor("ones_c_d", [DH, 1], F32R, kind="ExternalInput")
    ones_cb_d = nc.dram_tensor("ones_cb_d", [DH, 1], BF16, kind="ExternalInput")
    ones_s_d = nc.dram_tensor("ones_s_d", [1, DH], BF16, kind="ExternalInput")
    ones_r_d = nc.dram_tensor("ones_r_d", [1, DH], F32R, kind="ExternalInput")
    resid_d = nc.dram_tensor("resid_d", [EH, S], BF16, kind="ExternalInput")
    outT_d = nc.dram_tensor("outT_d", [EH, S], BF16, kind="ExternalOutput")

    qT4 = qT8[:].rearrange("p (sc f j s) -> p sc f j s", sc=NPC, f=NDR, j=2)
    kT4 = kT8[:].rearrange("p (sc f j s) -> p sc f j s", sc=NPC, f=NDR, j=2)

    with TileContext(nc) as tc, nc.allow_low_precision(reason="fp8/bf16 by design"):
        with tc.tile_pool(name="persist", bufs=1) as persist:
            tri = persist.tile([DH, DH], BF16, tag="tri", name="tri")
            kbias = persist.tile([DH, NKT], F32, tag="kbias", name="kbias")
            ones_c = persist.tile([DH, 1], F32R, tag="ones_c", name="ones_c")
            ones_cb = persist.tile([DH, 1], BF16, tag="ones_cb", name="ones_cb")
            ones_s = persist.tile([1, DH], BF16, tag="ones_s", name="ones_s")
            ones_r = persist.tile([1, DH], F32R, tag="ones_r", name="ones_r")
            bq_sb = persist.tile([DH, HPC], F32, tag="bq", name="bq_sb")
            bk_sb = persist.tile([DH, HPC], F32, tag="bk", name="bk_sb")
            bv_sb = persist.tile([1, EH], BF16, tag="bv", name="bv_sb")
            # tiny persistent loads go on the gpsimd queue so they don't
            # delay the weight/chunk DMAs on SP
            nc.gpsimd.dma_start(bk_sb[:], bk_d[:])
            nc.gpsimd.dma_start(bq_sb[:], bq_d[:])
            nc.gpsimd.dma_start(bv_sb[:], bv_d[:])
            nc.gpsimd.dma_start(tri[:], tri_d[:])
            nc.gpsimd.dma_start(kbias[:], kbias_d[:])
            nc.gpsimd.dma_start(ones_c[:], ones_c_d[:])
            nc.gpsimd.dma_start(ones_cb[:], ones_cb_d[:])
            nc.gpsimd.dma_start(ones_s[:], ones_s_d[:])
            nc.gpsimd.dma_start(ones_r[:], ones_r_d[:])

            # persistent per-head K/Q fp8 pair tiles + V bf16
            kt8_t = [
                persist.tile([64, 2 * S], F8, tag=f"kt8_{h}", name=f"kt8_{h}") for h in range(HPC)
            ]
            qt8_t = [
                persist.tile([64, 2 * S], F8, tag=f"qt8_{h}", name=f"qt8_{h}") for h in range(HPC)
            ]
            vfull = persist.tile([DH, NKT * EH], BF16, tag="vfull", name="vfull")

            body = _fused_body if opts["fused"] else _rep_body
            for _rep in range(reps):
                body(
                    nc, tc, phases, scale, fast, opts,
                    kT4, qT4, wk8, wv8, wq8,
                    kt8_t, qt8_t, vfull, resid_d, outT_d,
                    tri, kbias, (ones_c, ones_cb), ones_s, ones_r,
                    bq_sb, bk_sb, bv_sb,
                    F32, F32R, F8, BF16, EXP, IDENT, mybir,
                )

    _split_excess_waits(nc, max_waits=1, compute_max_waits=COMPUTE_MAX_WAITS)
    _CACHE[key] = nc
    return nc


def _rep_body(
    nc, tc, phases, scale, fast, opts,
    kT4, qT4, wk8, wv8, wq8,
    kt8_t, qt8_t, vfull, resid_d, outT_d,
    tri, kbias, ones_c, ones_s, ones_r,
    bq_sb, bk_sb, bv_sb,
    F32, F32R, F8, BF16, EXP, IDENT, mybir,
):
    import contextlib

    DR = mybir.MatmulPerfMode.DoubleRow

    # ---------------- Phase KV (fused K + V projection) ----------------
    if "kv" in phases:
        with contextlib.ExitStack() as st:
            wkp = st.enter_context(tc.tile_pool(name="wk", bufs=1))
            wvp = st.enter_context(tc.tile_pool(name="wv", bufs=1))
            kcp = st.enter_context(tc.tile_pool(name="kc", bufs=opts.get("ckb", 2)))
            pkp = st.enter_context(tc.tile_pool(name="pk", bufs=3, space="PSUM"))
            pvp = st.enter_context(tc.tile_pool(name="pv", bufs=3, space="PSUM"))
            wk_t = wkp.tile([DH, NDR * 2 * EH], F8, tag="wk", name="wk")
            nc.sync.dma_start(wk_t[:], wk8[:])
            wv_t = wvp.tile([DH, NDR * 2 * EH], F8, tag="wv", name="wv")
            nc.sync.dma_start(wv_t[:], wv8[:])
            wk3 = wk_t[:].rearrange("p (f j e) -> p f j e", f=NDR, j=2)
            wv3 = wv_t[:].rearrange("p (f j e) -> p f j e", f=NDR, j=2)
            for sc in range(NPC * scale.get("kv", 1)):
                s0 = (sc % NPC) * PC
                kc = kcp.tile([DH, NDR * 2 * PC], F8, tag="kc", name="kc")
                kc3 = kc[:].rearrange("p (f j s) -> p f j s", f=NDR, j=2)
                nc.sync.dma_start(kc3, kT4[:, sc % NPC])
                # K: per head, contract over f; drain to fp8 pair tiles
                for et in range(HPC):
                    pk = pkp.tile([DH, PC], F32, name="pk")
                    for f in range(NDR):
                        nc.tensor.matmul(
                            pk[:],
                            wk3[:, f, :, et * DH : (et + 1) * DH],
                            kc3[:, f],
                            start=(f == 0),
                            stop=(f == NDR - 1),
                            perf_mode=DR,
                        )
                    kt3 = kt8_t[et][:].rearrange("p (j s) -> p j s", j=2)
                    for j in range(2):
                        if opts["drains"] == "dve":
                            nc.vector.tensor_scalar(
                                kt3[:, j, s0 : s0 + PC],
                                pk[j * 64 : (j + 1) * 64, :],
                                bk_sb[j * 64 : (j + 1) * 64, et : et + 1],
                                None,
                                mybir.AluOpType.add,
                            )
                        else:
                            nc.scalar.activation(
                                kt3[:, j, s0 : s0 + PC],
                                pk[j * 64 : (j + 1) * 64, :],
                                IDENT,
                                bias=bk_sb[j * 64 : (j + 1) * 64, et : et + 1],
                            )
                # V: out[s, e]; stationary = K chunk s-tile, moving = wv
                for sti in range(PC // DH):
                    st_g = (sc % NPC) * (PC // DH) + sti
                    for ec in range(EH // QC):
                        pv = pvp.tile([DH, QC], F32, name="pv")
                        for f in range(NDR):
                            nc.tensor.matmul(
                                pv[:],
                                kc3[:, f, :, sti * DH : (sti + 1) * DH],
                                wv3[:, f, :, ec * QC : (ec + 1) * QC],
                                start=(f == 0),
                                stop=False,
                                perf_mode=DR,
                            )
                        nc.tensor.matmul(
                            pv[:],
                            ones_s[:],
                            bv_sb[:, ec * QC : (ec + 1) * QC],
                            start=False,
                            stop=True,
                        )
                        nc.vector.tensor_copy(
                            vfull[:, st_g * EH + ec * QC : st_g * EH + (ec + 1) * QC],
                            pv[:],
                        )

    # ---------------- Phase Q ----------------
    if "q" in phases:
        with contextlib.ExitStack() as st:
            wqp = st.enter_context(tc.tile_pool(name="wq", bufs=1))
            qcp = st.enter_context(tc.tile_pool(name="qc", bufs=opts.get("ckb", 2)))
            pqp = st.enter_context(tc.tile_pool(name="pq", bufs=4, space="PSUM"))
            wq_t = wqp.tile([DH, NDR * 2 * EH], F8, tag="wq", name="wq")
            nc.sync.dma_start(wq_t[:], wq8[:])
            wq3 = wq_t[:].rearrange("p (f j e) -> p f j e", f=NDR, j=2)
            for sc in range(NPC * scale.get("q", 1)):
                s0 = (sc % NPC) * PC
                qch = qcp.tile([DH, NDR * 2 * PC], F8, tag="qch", name="qch")
                qc3 = qch[:].rearrange("p (f j s) -> p f j s", f=NDR, j=2)
                nc.sync.dma_start(qc3, qT4[:, sc % NPC])
                for et in range(HPC):
                    pq = pqp.tile([DH, PC], F32, name="pq")
                    for f in range(NDR):
                        nc.tensor.matmul(
                            pq[:],
                            wq3[:, f, :, et * DH : (et + 1) * DH],
                            qc3[:, f],
                            start=(f == 0),
                            stop=(f == NDR - 1),
                            perf_mode=DR,
                        )
                    qt3 = qt8_t[et][:].rearrange("p (j s) -> p j s", j=2)
                    for j in range(2):
                        if opts["drains"] == "dve":
                            nc.vector.tensor_scalar(
                                qt3[:, j, s0 : s0 + PC],
                                pq[j * 64 : (j + 1) * 64, :],
                                bq_sb[j * 64 : (j + 1) * 64, et : et + 1],
                                None,
                                mybir.AluOpType.add,
                            )
                        else:
                            nc.scalar.activation(
                                qt3[:, j, s0 : s0 + PC],
                                pq[j * 64 : (j + 1) * 64, :],
                                IDENT,
                                bias=bq_sb[j * 64 : (j + 1) * 64, et : et + 1],
                            )

    # ---------------- Phase attention ----------------
    if "attn" in phases:
        _attention(
            nc, tc, fast, opts, kt8_t, qt8_t, vfull, resid_d, outT_d,
            tri, kbias, ones_c, ones_r, F32, F32R, BF16, EXP, mybir,
            scale.get("attn", 1),
        )


def _attention(
    nc, tc, fast, opts, kt8_t, qt8_t, vfull, resid_d, outT_d,
    tri, kbias, ones_c, ones_r, F32, F32R, BF16, EXP, mybir, attn_scale=1,
):
    ones_c, ones_cb = ones_c
    DR = mybir.MatmulPerfMode.DoubleRow
    with tc.tile_pool(name="ex", bufs=4) as exp_p, \
         tc.tile_pool(name="exs", bufs=2) as exsp, \
         tc.tile_pool(name="tail", bufs=2) as tailp, \
         tc.tile_pool(name="rsd", bufs=2) as rsdp, \
         tc.tile_pool(name="outs", bufs=2) as outp_sb, \
         tc.tile_pool(name="ps_s", bufs=3, space="PSUM") as pss, \
         tc.tile_pool(name="ps_o", bufs=2, space="PSUM") as pso, \
         tc.tile_pool(name="ps_t", bufs=1, space="PSUM") as pst, \
         tc.tile_pool(name="ps_b", bufs=1, space="PSUM") as psb:
        for hh in range(HPC * attn_scale):
            h = hh % HPC
            kt3 = kt8_t[h][:].rearrange("p (j s) -> p j s", j=2)
            qt3 = qt8_t[h][:].rearrange("p (j s) -> p j s", j=2)
            rsd = rsdp.tile([DH, S], BF16, tag="rsd", name="rsd")
            nc.sync.dma_start(rsd[:], resid_d[h * DH : (h + 1) * DH, :])
            oth = outp_sb.tile([DH, S], BF16, tag="oth", name="oth")
            for qc in range(NQC):
                q0 = qc * QC
                nkt = 4 * qc + 4
                nfull = 4 * qc  # full (non-band) k-tiles
                po = pso.tile([DH, QC], F32, name="po")
                use_pool = opts["rowsum"] == "pool"
                psum = pst.tile([1, QC], F32, name="psum")
                if use_pool:
                    exsum = exsp.tile([DH, QC], F32R, tag="exsum", name="exsum")

                for kt in range(nkt):
                    off = 0 if kt < nfull else (kt - nfull) * DH
                    ps = pss.tile([DH, QC], F32, name="ps")
                    nc.tensor.matmul(
                        ps[:, off:QC],
                        kt3[:, :, kt * DH : (kt + 1) * DH],
                        qt3[:, :, q0 + off : q0 + QC],
                        start=True,
                        stop=True,
                        perf_mode=DR,
                    )
                    ex = exp_p.tile([DH, QC], BF16, tag="ex", name="ex")
                    if fast:
                        nc.scalar.activation(
                            ex[:, off:QC], ps[:, off:QC], EXP, scale=float(SCALE)
                        )
                    else:
                        nc.scalar.activation(
                            ex[:, off:QC], ps[:, off:QC], EXP,
                            bias=kbias[:, kt : kt + 1], scale=float(SCALE),
                        )
                    if kt >= nfull:
                        # causal triangle on the diagonal 128x128 block
                        nc.vector.tensor_mul(
                            ex[:, off : off + DH], ex[:, off : off + DH], tri[:]
                        )
                    if use_pool:
                        # row-sum accumulate on the (otherwise idle) Pool engine
                        if kt == 0:
                            nc.gpsimd.tensor_copy(exsum[:, off:QC], ex[:, off:QC])
                        else:
                            nc.gpsimd.tensor_add(
                                exsum[:, off:QC], exsum[:, off:QC], ex[:, off:QC]
                            )
                    else:
                        nc.tensor.matmul(
                            psum[:, off:QC],
                            ones_cb[:],
                            ex[:, off:QC],
                            start=(kt == 0),
                            stop=(kt == nkt - 1),
                        )
                    nc.tensor.matmul(
                        po[:, off:QC],
                        vfull[:, kt * EH + h * DH : kt * EH + (h + 1) * DH],
                        ex[:, off:QC],
                        start=(kt == 0),
                        stop=(kt == nkt - 1),
                    )
                if use_pool:
                    nc.tensor.matmul(
                        psum[:], ones_c[:], exsum[:], start=True, stop=True
                    )
                rec = tailp.tile([1, QC], F32R, tag="rec", name="rec")
                nc.vector.reciprocal(rec[:], psum[:])
                pbc = psb.tile([DH, QC], F32, name="pbc")
                nc.tensor.matmul(pbc[:], ones_r[:], rec[:], start=True, stop=True)
                bcs = tailp.tile([DH, QC], F32, tag="bcs", name="bcs")
                nc.vector.tensor_copy(bcs[:], pbc[:])
                nc.vector.tensor_mul(bcs[:], po[:], bcs[:])
                nc.vector.tensor_add(
                    oth[:, q0 : q0 + QC], bcs[:], rsd[:, q0 : q0 + QC]
                )
            nc.sync.dma_start(outT_d[h * DH : (h + 1) * DH, :], oth[:])


def _host_prep(queries, keys, Wq, bq, Wk, bk, Wv, bv):
    """Build the 8 per-core input maps (host-side shard + layout prep)."""
    import ml_dtypes

    f8 = ml_dtypes.float8_e4m3
    bf16 = ml_dtypes.bfloat16
    queries = np.ascontiguousarray(queries, dtype=np.float32)
    keys = np.ascontiguousarray(keys, dtype=np.float32)

    def pair_chunks(x):
        # x [S, HID] -> [128p, (sc, f, j, s')] with hid = f*256 + j*128 + p
        a = x.T.reshape(NDR, 2, DH, S)          # [f, j, p, s]
        a = a.transpose(2, 3, 0, 1)             # [p, s, f, j]
        a = a.reshape(DH, NPC, PC, NDR, 2)      # [p, sc, s', f, j]
        a = a.transpose(0, 1, 3, 4, 2)          # [p, sc, f, j, s']
        return np.ascontiguousarray(a.astype(f8)).reshape(DH, -1)

    def pair_weights(w, e0):
        # w [HID(out), HID(in)] -> [128p, (et, f, j, dh)] for out slice
        # e0:e0+EH (et-major so K-proj group et only needs piece et)
        a = w[e0 : e0 + EH, :].T                # [hid, e]
        a = a.reshape(NDR, 2, DH, HPC, DH)      # [f, j, p, et, dh]
        a = a.transpose(2, 3, 0, 1, 4)          # [p, et, f, j, dh]
        return np.ascontiguousarray(a.astype(f8)).reshape(DH, -1)

    qT8_b = [pair_chunks(queries[b]) for b in range(B)]
    kT8_b = [pair_chunks(keys[b]) for b in range(B)]
    residT = [
        np.ascontiguousarray(queries[b].T.astype(bf16)) for b in range(B)
    ]

    Wq = np.asarray(Wq, np.float32)
    Wk = np.asarray(Wk, np.float32)
    Wv = np.asarray(Wv, np.float32)
    bq = np.asarray(bq, np.float32)
    bk = np.asarray(bk, np.float32)
    bv = np.asarray(bv, np.float32)

    ksum = keys.sum(axis=-1)  # [B, S]
    kbias_all = np.where(ksum != 0.0, np.float32(0), NEG_BIAS).astype(np.float32)

    tri = (np.arange(DH)[None, :] >= np.arange(DH)[:, None]).astype(bf16)
    ones_c = np.ones((DH, 1), np.float32)
    ones_cb = np.ones((DH, 1), bf16)
    ones_s = np.ones((1, DH), bf16)
    ones_r = np.ones((1, DH), np.float32)

    in_maps = []
    for c in range(NCORES):
        b, hg = divmod(c, 2)
        e0 = hg * EH
        in_maps.append(
            {
                "qT8": qT8_b[b],
                "kT8": kT8_b[b],
                "wq8": pair_weights(Wq, e0),
                "wk8": pair_weights(Wk, e0),
                "wv8": pair_weights(Wv, e0),
                "bq_d": np.ascontiguousarray(bq[e0 : e0 + EH].reshape(HPC, DH).T),
                "bk_d": np.ascontiguousarray(bk[e0 : e0 + EH].reshape(HPC, DH).T),
                "bv_d": np.ascontiguousarray(
                    bv[e0 : e0 + EH].reshape(1, EH)
                ).astype(bf16),
                "kbias_d": np.ascontiguousarray(kbias_all[b].reshape(NKT, DH).T),
                "tri_d": tri,
                "ones_c_d": ones_c,
                "ones_cb_d": ones_cb,
                "ones_s_d": ones_s,
                "ones_r_d": ones_r,
                "resid_d": residT[b][e0 : e0 + EH, :],
            }
        )
    return in_maps


def _assemble(results):
    """results: list of 8 dicts with outT_d [EH, S] bf16 -> full [B, S, HID]."""
    out = np.empty((B, S, HID), np.float32)
    for c in range(NCORES):
        b, hg = divmod(c, 2)
        out[b, :, hg * EH : (hg + 1) * EH] = results[c]["outT_d"].T.astype(
            np.float32
        )
    return out


def kernel(**inputs):
    from concourse.bass_utils import run_bass_kernel_spmd

    # fast path is valid unless some key row is exactly zero-sum (padding)
    keys = np.asarray(inputs["keys"], np.float32)
    fast = not bool(np.any(keys.sum(axis=-1) == 0.0))
    nc = _build(fast=fast)
    in_maps = _host_prep(**inputs)
    res = run_bass_kernel_spmd(nc, in_maps, core_ids=list(range(NCORES)))
    kernel.last_results = res
    return _assemble(res.results)


def _fused_body(
    nc, tc, phases, scale, fast, opts,
    kT4, qT4, wk8, wv8, wq8,
    kt8_t, qt8_t, vfull, resid_d, outT_d,
    tri, kbias, ones_c, ones_s, ones_r,
    bq_sb, bk_sb, bv_sb,
    F32, F32R, F8, BF16, EXP, IDENT, mybir,
):
    """Chunk-level pipeline: for each s-chunk sc, project K/V/Q(sc) then run
    attention for q-chunk sc over all heads; chunk sc+1's projections overlap
    attention sc on disjoint engines/PSUM banks."""
    import contextlib

    ones_c, ones_cb = ones_c
    DR = mybir.MatmulPerfMode.DoubleRow

    with contextlib.ExitStack() as st:
        wkp = st.enter_context(tc.tile_pool(name="wk", bufs=1))
        wvp = st.enter_context(tc.tile_pool(name="wv", bufs=1))
        wqp = st.enter_context(tc.tile_pool(name="wq", bufs=1))
        kcp = st.enter_context(tc.tile_pool(name="kc", bufs=opts.get("ckb", 2)))
        qcp = st.enter_context(tc.tile_pool(name="qc", bufs=opts.get("ckb", 2)))
        ppp = st.enter_context(
            tc.tile_pool(name="pp", bufs=opts.get("pp", 2), space="PSUM"))
        exp_p = st.enter_context(tc.tile_pool(name="ex", bufs=opts.get("exb", 4)))
        tailp = st.enter_context(tc.tile_pool(name="tail", bufs=opts.get("tlb", 2)))
        rsdp = st.enter_context(tc.tile_pool(name="rsd", bufs=3))
        outp = st.enter_context(tc.tile_pool(name="outs", bufs=3))
        bc_pool = opts.get("bcast", "pool") == "pool"
        nss = opts.get("pss", 2)
        npp = opts.get("pp", 2)
        nso = 8 - npp - nss - 1 - (0 if bc_pool else 1)
        pss = st.enter_context(tc.tile_pool(name="ps_s", bufs=nss, space="PSUM"))
        pso = st.enter_context(tc.tile_pool(name="ps_o", bufs=nso, space="PSUM"))
        pst = st.enter_context(tc.tile_pool(name="ps_t", bufs=1, space="PSUM"))
        psb = (None if bc_pool else
               st.enter_context(tc.tile_pool(name="ps_b", bufs=1, space="PSUM")))

        ETW = NDR * 2 * DH  # per-et weight block (columns)
        wk_t = wkp.tile([DH, NDR * 2 * EH], F8, tag="wk", name="wk_t")
        wv_t = wvp.tile([DH, NDR * 2 * EH], F8, tag="wv", name="wv_t")
        wq_t = wqp.tile([DH, NDR * 2 * EH], F8, tag="wq", name="wq_t")
        # wk in per-et pieces: K-proj group et starts after its piece lands
        nwkp = opts.get("wk_pieces", 1)
        WPC = NDR * 2 * EH // nwkp
        for i in range(nwkp):
            nc.sync.dma_start(
                wk_t[:, i * WPC : (i + 1) * WPC],
                wk8[:, i * WPC : (i + 1) * WPC])
        # wv/wq halves on the Activation hwdge queue so they don't delay
        # the first K chunk behind them on SP
        HW8 = NDR * 2 * EH // 2
        for lo in (0, HW8):
            nc.scalar.dma_start(wv_t[:, lo : lo + HW8], wv8[:, lo : lo + HW8])
        for lo in (0, HW8):
            nc.scalar.dma_start(wq_t[:, lo : lo + HW8], wq8[:, lo : lo + HW8])
        # [p, et, f, j, dh]
        wk4 = wk_t[:].rearrange("p (et f j d) -> p et f j d", et=HPC, f=NDR, j=2)
        wv4 = wv_t[:].rearrange("p (et f j d) -> p et f j d", et=HPC, f=NDR, j=2)
        wq4 = wq_t[:].rearrange("p (et f j d) -> p et f j d", et=HPC, f=NDR, j=2)

        for sc in range(NPC):
            s0 = sc * PC
            # ---- K/V/Q projections for chunk sc ----
            kc = kcp.tile([DH, NDR * 2 * PC], F8, tag="kc", name="kc")
            kc3 = kc[:].rearrange("p (f j s) -> p f j s", f=NDR, j=2)
            nc.sync.dma_start(kc3, kT4[:, sc])
            qch = qcp.tile([DH, NDR * 2 * PC], F8, tag="qch", name="qch")
            qc3 = qch[:].rearrange("p (f j s) -> p f j s", f=NDR, j=2)
            nc.sync.dma_start(qc3, qT4[:, sc])
            for et in range(HPC):
                pk = ppp.tile([DH, PC], F32, tag="pp", name="pk")
                for f in range(NDR):
                    nc.tensor.matmul(
                        pk[:], wk4[:, et, f], kc3[:, f],
                        start=(f == 0), stop=(f == NDR - 1), perf_mode=DR)
                kt3 = kt8_t[et][:].rearrange("p (j s) -> p j s", j=2)
                for j in range(2):
                    dv = opts["drains"] == "dve" or (
                        opts["drains"] == "mix" and j == 1) or (
                        opts.get("last_dve") and sc == NPC - 1)
                    if dv:
                        nc.vector.tensor_scalar(
                            kt3[:, j, s0 : s0 + PC],
                            pk[j * 64 : (j + 1) * 64, :],
                            bk_sb[j * 64 : (j + 1) * 64, et : et + 1],
                            None, mybir.AluOpType.add)
                    else:
                        nc.scalar.activation(
                            kt3[:, j, s0 : s0 + PC],
                            pk[j * 64 : (j + 1) * 64, :], IDENT,
                            bias=bk_sb[j * 64 : (j + 1) * 64, et : et + 1])
            for sti in range(PC // DH):
                st_g = sc * (PC // DH) + sti
                for ec in range(EH // QC):
                    pv = ppp.tile([DH, QC], F32, tag="pp", name="pv")
                    nhe = QC // DH  # et blocks per e-chunk
                    for f in range(NDR):
                        wv_r = wv4[:, ec * nhe : (ec + 1) * nhe, f].rearrange(
                            "p et j d -> p j et d")
                        nc.tensor.matmul(
                            pv[:],
                            kc3[:, f, :, sti * DH : (sti + 1) * DH],
                            wv_r,
                            start=(f == 0), stop=False, perf_mode=DR)
                    nc.tensor.matmul(
                        pv[:], ones_s[:], bv_sb[:, ec * QC : (ec + 1) * QC],
                        start=False, stop=True)
                    if opts.get("vdrain", "dve") == "pool":
                        nc.gpsimd.tensor_copy(
                            vfull[:, st_g * EH + ec * QC : st_g * EH + (ec + 1) * QC],
                            pv[:])
                    else:
                        nc.vector.tensor_copy(
                            vfull[:, st_g * EH + ec * QC : st_g * EH + (ec + 1) * QC],
                            pv[:])
            for et in range(HPC):
                pq = ppp.tile([DH, PC], F32, tag="pp", name="pq")
                for f in range(NDR):
                    nc.tensor.matmul(
                        pq[:], wq4[:, et, f], qc3[:, f],
                        start=(f == 0), stop=(f == NDR - 1), perf_mode=DR)
                qt3 = qt8_t[et][:].rearrange("p (j s) -> p j s", j=2)
                for j in range(2):
                    dv = opts["drains"] == "dve" or (
                        opts["drains"] == "mix" and j == 1) or (
                        opts.get("last_dve") and sc == NPC - 1)
                    if dv:
                        nc.vector.tensor_scalar(
                            qt3[:, j, s0 : s0 + PC],
                            pq[j * 64 : (j + 1) * 64, :],
                            bq_sb[j * 64 : (j + 1) * 64, et : et + 1],
                            None, mybir.AluOpType.add)
                    else:
                        nc.scalar.activation(
                            qt3[:, j, s0 : s0 + PC],
                            pq[j * 64 : (j + 1) * 64, :], IDENT,
                            bias=bq_sb[j * 64 : (j + 1) * 64, et : et + 1])

            # ---- attention for q-chunk sc, all heads ----
            qc_ = sc
            q0 = qc_ * QC
            nkt = 4 * qc_ + 4
            nfull = 4 * qc_
            use_pool = opts["rowsum"] == "pool"
            for h in range(HPC):
                kt3 = kt8_t[h][:].rearrange("p (j s) -> p j s", j=2)
                qt3 = qt8_t[h][:].rearrange("p (j s) -> p j s", j=2)
                rsd = rsdp.tile([DH, QC], BF16, tag="rsd", name="rsd")
                nc.gpsimd.dma_start(
                    rsd[:], resid_d[h * DH : (h + 1) * DH, q0 : q0 + QC])
                po = pso.tile([DH, QC], F32, name="po")
                psum = pst.tile([1, QC], F32, name="psum")
                if use_pool:
                    exsum = tailp.tile([DH, QC], F32R, tag="exsum", name="exsum")
                for kt in range(nkt):
                    off = 0 if kt < nfull else (kt - nfull) * DH
                    ps = pss.tile([DH, QC], F32, name="ps")
                    nc.tensor.matmul(
                        ps[:, off:QC],
                        kt3[:, :, kt * DH : (kt + 1) * DH],
                        qt3[:, :, q0 + off : q0 + QC],
                        start=True, stop=True, perf_mode=DR)
                    ex = exp_p.tile([DH, QC], BF16, tag="ex", name="ex")
                    if fast:
                        nc.scalar.activation(
                            ex[:, off:QC], ps[:, off:QC], EXP, scale=float(SCALE))
                    else:
                        nc.scalar.activation(
                            ex[:, off:QC], ps[:, off:QC], EXP,
                            bias=kbias[:, kt : kt + 1], scale=float(SCALE))
                    if kt >= nfull:
                        if opts.get("tri", "dve") == "pool":
                            nc.gpsimd.tensor_mul(
                                ex[:, off : off + DH], ex[:, off : off + DH],
                                tri[:])
                        else:
                            nc.vector.tensor_mul(
                                ex[:, off : off + DH], ex[:, off : off + DH],
                                tri[:])
                    if use_pool:
                        if kt == 0:
                            nc.gpsimd.tensor_copy(exsum[:, off:QC], ex[:, off:QC])
                        else:
                            nc.gpsimd.tensor_add(
                                exsum[:, off:QC], exsum[:, off:QC], ex[:, off:QC])
                    else:
                        nc.tensor.matmul(
                            psum[:, off:QC], ones_cb[:], ex[:, off:QC],
                            start=(kt == 0), stop=(kt == nkt - 1))
                    nc.tensor.matmul(
                        po[:, off:QC],
                        vfull[:, kt * EH + h * DH : kt * EH + (h + 1) * DH],
                        ex[:, off:QC],
                        start=(kt == 0), stop=(kt == nkt - 1))
                if use_pool:
                    nc.tensor.matmul(
                        psum[:], ones_c[:], exsum[:], start=True, stop=True)
                rec = tailp.tile([1, QC], F32R, tag="rec", name="rec")
                nc.vector.reciprocal(rec[:], psum[:])
                bcs = tailp.tile(
                    [DH, QC], BF16 if opts.get("bcs16") else F32R,
                    tag="bcs", name="bcs")
                if bc_pool:
                    nc.gpsimd.partition_broadcast(bcs[:], rec[:])
                else:
                    pbc = psb.tile([DH, QC], F32, name="pbc")
                    nc.tensor.matmul(
                        pbc[:], ones_r[:], rec[:], start=True, stop=True)
                    nc.vector.tensor_copy(bcs[:], pbc[:])
                nc.vector.tensor_mul(bcs[:], po[:], bcs[:])
                oth = outp.tile([DH, QC], BF16, tag="oth", name="oth")
                if opts.get("radd", "dve") == "pool":
                    nc.gpsimd.tensor_add(oth[:], bcs[:], rsd[:])
                else:
                    nc.vector.tensor_add(oth[:], bcs[:], rsd[:])
                nc.gpsimd.dma_start(
                    outT_d[h * DH : (h + 1) * DH, q0 : q0 + QC], oth[:])


# revision 6
# speedup vs baseline: 1.9598x; 1.9598x over previous
"""Trainium2 Bass kernel for nn_MultiHeadAttention_88210038326473 (v2, fp8).

Reference computation (B=4, S=2048, HID=2048, H=16, DH=128):
    Q = queries @ Wq.T + bq ; K = keys @ Wk.T + bk ; V = keys @ Wv.T + bv
    per-head scores = Qh Kh^T / sqrt(HID), key-padding + causal mask,
    softmax, out = attn @ Vh, concat heads, + queries residual.

Sharding: 8 cores = 4 batches x 2 head-groups (8 heads each). Each core
computes out[b, :, hg*1024:(hg+1)*1024] (stored transposed [1024, 2048]
bf16; host transposes back and assembles).

v2 design (all SBUF-resident, fp8 DoubleRow matmuls, fused pipeline):
  - Projections contract HID=2048 as 8 fp8-DoubleRow matmuls of K=256
    (pair layout hid = f*256 + j*128 + p). Inputs kT8/qT8 and weights
    (et-major) are pre-quantized e4m3 on host. fp8 inputs/weights +
    fp8 Q/K for scores measure 4e-3 rel err vs the 2e-2 budget.
  - K and Q projections write fp8 pair-layout tiles kt8/qt8 [64,(2,S)]
    per head (d = j*64 + p), so scores are fp8 DoubleRow too.
  - V projection writes bf16 vfull [128,(st,e)] (partition = s%128).
  - Scores: sT[k,q] DoubleRow per 128-k-tile into PSUM fp32; exp on the
    scalar engine (no max subtraction: scores are O(1)); probabilities
    stay bf16 (fp8 probs would breach the 2e-2 budget); causal diagonal
    via a 0/1 triangle multiply on DVE; attn@V in bf16; row sums via a
    ones-column matmul per k-tile; reciprocal + ones-row broadcast
    matmul; normalize on DVE + bf16 residual added on the Pool engine.
  - Chunk-level pipeline: for each 512-row s-chunk, project K(sc),
    Q(sc), V(sc) then run attention for q-chunk sc over all heads, so
    chunk sc+1's projections overlap attention sc across engines.
    Projection psum drains alternate Activation/DVE ("mix"); the last
    chunk's drains go to DVE to keep Activation free for the exp-heavy
    tail. TimelineSim: 309us vs 624us for the bf16 phased baseline.
"""

import math

import numpy as np

B, S, HID, H, DH = 4, 2048, 2048, 16, 128
NCORES = 8
HPC = 8          # heads per core
EH = HPC * DH    # 1024 e-dims per core
SCALE = 1.0 / math.sqrt(HID)
QC = 512         # attention q-chunk
NQC = S // QC    # 4
NKT = S // DH    # 16 k-tiles
NDR = HID // 256  # 8 DoubleRow contraction tiles
PC = 512         # projection s-chunk
NPC = S // PC    # 4
NEG_BIAS = np.float32(-1.0e30)
COMPUTE_MAX_WAITS = 1


CTRL_OPS = ("InstDrain", "InstNoOp", "InstEventSemaphore", "InstISA")


def _split_excess_waits(nc, max_waits=1, compute_max_waits=None):
    """walrus in this container rejects >1 sem-wait per CTRL-class instruction.
    Move excess waits onto preceding NoOps on the same engine."""
    import concourse.mybir as mybir

    if compute_max_waits is None:
        compute_max_waits = max_waits
    n_split = 0
    for fn in nc.m.functions:
        for blk in fn.blocks:
            insts = list(blk.instructions)
            out = []
            changed = False
            for ins in insts:
                lim = (
                    max_waits
                    if type(ins).__name__ in CTRL_OPS
                    else compute_max_waits
                )
                si = ins.sync_info
                if si is not None and si.on_wait and len(si.on_wait) > lim:
                    waits = list(si.on_wait)
                    carriers, rest = waits[:-lim], waits[-lim:]
                    for i in range(0, len(carriers), max_waits):
                        chunk = carriers[i : i + max_waits]
                        out.append(
                            mybir.InstNoOp(
                                name=f"{ins.name}-ws{i}",
                                engine=ins.engine,
                                bass_nofuse=True,
                                sync_info=mybir.SyncInfo(on_wait=chunk, on_update=[]),
                            )
                        )
                        n_split += 1
                    ins.sync_info = mybir.SyncInfo(
                        on_wait=rest, on_update=list(si.on_update)
                    )
                    changed = True
                out.append(ins)
            if changed:
                blk.instructions = out
    return n_split


_CACHE = {}


def _build(fast=True, phases=("kv", "q", "attn"), reps=1, scale=None, opts=None):
    """Build the (core-uniform) Bass program. Returns nc."""
    scale = scale or {}
    opts = dict(opts or {})
    opts.setdefault("drains", "mix")   # "act" | "dve" | "mix"
    opts.setdefault("rowsum", "pe")    # "pe" | "pool"
    opts.setdefault("bcast", "pe")     # "pe" | "pool"
    opts.setdefault("last_dve", True)  # route last-chunk drains to DVE
    opts.setdefault("pp", 2)
    opts.setdefault("pss", 3)
    opts.setdefault("radd", "pool")
    opts.setdefault("qv_order", "qv")
    opts.setdefault("fused", True)     # chunk-level KV+Q+attn pipeline
    key = ("nc2", fast, tuple(phases), reps, tuple(sorted(scale.items())),
           tuple(sorted(opts.items())))
    if key in _CACHE:
        return _CACHE[key]

    import concourse.bass as bass
    import concourse.mybir as mybir
    from concourse.tile import TileContext

    F32 = mybir.dt.float32
    F32R = mybir.dt.float32r
    F8 = mybir.dt.float8e4
    BF16 = mybir.dt.bfloat16
    EXP = mybir.ActivationFunctionType.Exp
    IDENT = mybir.ActivationFunctionType.Identity

    nc = bass.Bass("TRN2", target_bir_lowering=False, debug=False)

    # inputs (pair layout hid = f*256 + j*128 + p; s-chunk-major for
    # contiguous chunk DMA)
    qT8 = nc.dram_tensor("qT8", [DH, NPC * NDR * 2 * PC], F8, kind="ExternalInput")
    kT8 = nc.dram_tensor("kT8", [DH, NPC * NDR * 2 * PC], F8, kind="ExternalInput")
    wq8 = nc.dram_tensor("wq8", [DH, NDR * 2 * EH], F8, kind="ExternalInput")
    wk8 = nc.dram_tensor("wk8", [DH, NDR * 2 * EH], F8, kind="ExternalInput")
    wv8 = nc.dram_tensor("wv8", [DH, NDR * 2 * EH], F8, kind="ExternalInput")
    bq_d = nc.dram_tensor("bq_d", [DH, HPC], F32, kind="ExternalInput")
    bk_d = nc.dram_tensor("bk_d", [DH, HPC], F32, kind="ExternalInput")
    bv_d = nc.dram_tensor("bv_d", [1, EH], BF16, kind="ExternalInput")
    kbias_d = nc.dram_tensor("kbias_d", [DH, NKT], F32, kind="ExternalInput")
    tri_d = nc.dram_tensor("tri_d", [DH, DH], BF16, kind="ExternalInput")
    ones_c_d = nc.dram_tensor("ones_c_d", [DH, 1], F32R, kind="ExternalInput")
    ones_cb_d = nc.dram_tensor("ones_cb_d", [DH, 1], BF16, kind="ExternalInput")
    ones_s_d = nc.dram_tensor("ones_s_d", [1, DH], BF16, kind="ExternalInput")
    ones_r_d = nc.dram_tensor("ones_r_d", [1, DH], F32R, kind="ExternalInput")
    resid_d = nc.dram_tensor("resid_d", [EH, S], BF16, kind="ExternalInput")
    outT_d = nc.dram_tensor("outT_d", [EH, S], BF16, kind="ExternalOutput")

    qT4 = qT8[:].rearrange("p (sc f j s) -> p sc f j s", sc=NPC, f=NDR, j=2)
    kT4 = kT8[:].rearrange("p (sc f j s) -> p sc f j s", sc=NPC, f=NDR, j=2)

    with TileContext(nc) as tc, nc.allow_low_precision(reason="fp8/bf16 by design"):
        with tc.tile_pool(name="persist", bufs=1) as persist:
            tri = persist.tile([DH, DH], BF16, tag="tri", name="tri")
            kbias = persist.tile([DH, NKT], F32, tag="kbias", name="kbias")
            ones_c = persist.tile([DH, 1], F32R, tag="ones_c", name="ones_c")
            ones_cb = persist.tile([DH, 1], BF16, tag="ones_cb", name="ones_cb")
            ones_s = persist.tile([1, DH], BF16, tag="ones_s", name="ones_s")
            ones_r = persist.tile([1, DH], F32R, tag="ones_r", name="ones_r")
            bq_sb = persist.tile([DH, HPC], F32, tag="bq", name="bq_sb")
            bk_sb = persist.tile([DH, HPC], F32, tag="bk", name="bk_sb")
            bv_sb = persist.tile([1, EH], BF16, tag="bv", name="bv_sb")
            # tiny persistent loads go on the gpsimd queue so they don't
            # delay the weight/chunk DMAs on SP
            nc.gpsimd.dma_start(bk_sb[:], bk_d[:])
            nc.gpsimd.dma_start(bq_sb[:], bq_d[:])
            nc.gpsimd.dma_start(bv_sb[:], bv_d[:])
            nc.gpsimd.dma_start(tri[:], tri_d[:])
            nc.gpsimd.dma_start(kbias[:], kbias_d[:])
            nc.gpsimd.dma_start(ones_c[:], ones_c_d[:])
            nc.gpsimd.dma_start(ones_cb[:], ones_cb_d[:])
            nc.gpsimd.dma_start(ones_s[:], ones_s_d[:])
            nc.gpsimd.dma_start(ones_r[:], ones_r_d[:])

            # persistent K/Q fp8 pair tiles, two heads packed per tile
            # (head 2i at partitions 0-63, head 2i+1 at 64-127)
            kt8_t = [
                persist.tile([DH, 2 * S], F8, tag=f"kt8_{i}", name=f"kt8_{i}")
                for i in range(HPC // 2)
            ]
            qt8_t = [
                persist.tile([DH, 2 * S], F8, tag=f"qt8_{i}", name=f"qt8_{i}")
                for i in range(HPC // 2)
            ]
            vfull = persist.tile([DH, NKT * EH], BF16, tag="vfull", name="vfull")

            body = _fused_body if opts["fused"] else _rep_body
            for _rep in range(reps):
                body(
                    nc, tc, phases, scale, fast, opts,
                    kT4, qT4, wk8, wv8, wq8,
                    kt8_t, qt8_t, vfull, resid_d, outT_d,
                    tri, kbias, (ones_c, ones_cb), ones_s, ones_r,
                    bq_sb, bk_sb, bv_sb,
                    F32, F32R, F8, BF16, EXP, IDENT, mybir,
                )

    _split_excess_waits(nc, max_waits=1, compute_max_waits=COMPUTE_MAX_WAITS)
    _CACHE[key] = nc
    return nc


def _rep_body(
    nc, tc, phases, scale, fast, opts,
    kT4, qT4, wk8, wv8, wq8,
    kt8_t, qt8_t, vfull, resid_d, outT_d,
    tri, kbias, ones_c, ones_s, ones_r,
    bq_sb, bk_sb, bv_sb,
    F32, F32R, F8, BF16, EXP, IDENT, mybir,
):
    import contextlib

    DR = mybir.MatmulPerfMode.DoubleRow

    # ---------------- Phase KV (fused K + V projection) ----------------
    if "kv" in phases:
        with contextlib.ExitStack() as st:
            wkp = st.enter_context(tc.tile_pool(name="wk", bufs=1))
            wvp = st.enter_context(tc.tile_pool(name="wv", bufs=1))
            kcp = st.enter_context(tc.tile_pool(name="kc", bufs=opts.get("ckb", 2)))
            pkp = st.enter_context(tc.tile_pool(name="pk", bufs=3, space="PSUM"))
            pvp = st.enter_context(tc.tile_pool(name="pv", bufs=3, space="PSUM"))
            wk_t = wkp.tile([DH, NDR * 2 * EH], F8, tag="wk", name="wk")
            nc.sync.dma_start(wk_t[:], wk8[:])
            wv_t = wvp.tile([DH, NDR * 2 * EH], F8, tag="wv", name="wv")
            nc.sync.dma_start(wv_t[:], wv8[:])
            wk3 = wk_t[:].rearrange("p (f j e) -> p f j e", f=NDR, j=2)
            wv3 = wv_t[:].rearrange("p (f j e) -> p f j e", f=NDR, j=2)
            for sc in range(NPC * scale.get("kv", 1)):
                s0 = (sc % NPC) * PC
                kc = kcp.tile([DH, NDR * 2 * PC], F8, tag="kc", name="kc")
                kc3 = kc[:].rearrange("p (f j s) -> p f j s", f=NDR, j=2)
                nc.sync.dma_start(kc3, kT4[:, sc % NPC])
                # K: per head, contract over f; drain to fp8 pair tiles
                for et in range(HPC):
                    pk = pkp.tile([DH, PC], F32, name="pk")
                    for f in range(NDR):
                        nc.tensor.matmul(
                            pk[:],
                            wk3[:, f, :, et * DH : (et + 1) * DH],
                            kc3[:, f],
                            start=(f == 0),
                            stop=(f == NDR - 1),
                            perf_mode=DR,
                        )
                    kt3 = kt8_t[et][:].rearrange("p (j s) -> p j s", j=2)
                    for j in range(2):
                        if opts["drains"] == "dve":
                            nc.vector.tensor_scalar(
                                kt3[:, j, s0 : s0 + PC],
                                pk[j * 64 : (j + 1) * 64, :],
                                bk_sb[j * 64 : (j + 1) * 64, et : et + 1],
                                None,
                                mybir.AluOpType.add,
                            )
                        else:
                            nc.scalar.activation(
                                kt3[:, j, s0 : s0 + PC],
                                pk[j * 64 : (j + 1) * 64, :],
                                IDENT,
                                bias=bk_sb[j * 64 : (j + 1) * 64, et : et + 1],
                            )
                # V: out[s, e]; stationary = K chunk s-tile, moving = wv
                for sti in range(PC // DH):
                    st_g = (sc % NPC) * (PC // DH) + sti
                    for ec in range(EH // QC):
                        pv = pvp.tile([DH, QC], F32, name="pv")
                        for f in range(NDR):
                            nc.tensor.matmul(
                                pv[:],
                                kc3[:, f, :, sti * DH : (sti + 1) * DH],
                                wv3[:, f, :, ec * QC : (ec + 1) * QC],
                                start=(f == 0),
                                stop=False,
                                perf_mode=DR,
                            )
                        nc.tensor.matmul(
                            pv[:],
                            ones_s[:],
                            bv_sb[:, ec * QC : (ec + 1) * QC],
                            start=False,
                            stop=True,
                        )
                        nc.vector.tensor_copy(
                            vfull[:, st_g * EH + ec * QC : st_g * EH + (ec + 1) * QC],
                            pv[:],
                        )

    # ---------------- Phase Q ----------------
    if "q" in phases:
        with contextlib.ExitStack() as st:
            wqp = st.enter_context(tc.tile_pool(name="wq", bufs=1))
            qcp = st.enter_context(tc.tile_pool(name="qc", bufs=opts.get("ckb", 2)))
            pqp = st.enter_context(tc.tile_pool(name="pq", bufs=4, space="PSUM"))
            wq_t = wqp.tile([DH, NDR * 2 * EH], F8, tag="wq", name="wq")
            nc.sync.dma_start(wq_t[:], wq8[:])
            wq3 = wq_t[:].rearrange("p (f j e) -> p f j e", f=NDR, j=2)
            for sc in range(NPC * scale.get("q", 1)):
                s0 = (sc % NPC) * PC
                qch = qcp.tile([DH, NDR * 2 * PC], F8, tag="qch", name="qch")
                qc3 = qch[:].rearrange("p (f j s) -> p f j s", f=NDR, j=2)
                nc.sync.dma_start(qc3, qT4[:, sc % NPC])
                for et in range(HPC):
                    pq = pqp.tile([DH, PC], F32, name="pq")
                    for f in range(NDR):
                        nc.tensor.matmul(
                            pq[:],
                            wq3[:, f, :, et * DH : (et + 1) * DH],
                            qc3[:, f],
                            start=(f == 0),
                            stop=(f == NDR - 1),
                            perf_mode=DR,
                        )
                    qt3 = _head_view(qt8_t, et)
                    for j in range(2):
                        if opts["drains"] == "dve":
                            nc.vector.tensor_scalar(
                                qt3[:, j, s0 : s0 + PC],
                                pq[j * 64 : (j + 1) * 64, :],
                                bq_sb[j * 64 : (j + 1) * 64, et : et + 1],
                                None,
                                mybir.AluOpType.add,
                            )
                        else:
                            nc.scalar.activation(
                                qt3[:, j, s0 : s0 + PC],
                                pq[j * 64 : (j + 1) * 64, :],
                                IDENT,
                                bias=bq_sb[j * 64 : (j + 1) * 64, et : et + 1],
                            )

    # ---------------- Phase attention ----------------
    if "attn" in phases:
        _attention(
            nc, tc, fast, opts, kt8_t, qt8_t, vfull, resid_d, outT_d,
            tri, kbias, ones_c, ones_r, F32, F32R, BF16, EXP, mybir,
            scale.get("attn", 1),
        )


def _attention(
    nc, tc, fast, opts, kt8_t, qt8_t, vfull, resid_d, outT_d,
    tri, kbias, ones_c, ones_r, F32, F32R, BF16, EXP, mybir, attn_scale=1,
):
    ones_c, ones_cb = ones_c
    DR = mybir.MatmulPerfMode.DoubleRow
    with tc.tile_pool(name="ex", bufs=4) as exp_p, \
         tc.tile_pool(name="exs", bufs=2) as exsp, \
         tc.tile_pool(name="tail", bufs=2) as tailp, \
         tc.tile_pool(name="rsd", bufs=2) as rsdp, \
         tc.tile_pool(name="outs", bufs=2) as outp_sb, \
         tc.tile_pool(name="ps_s", bufs=3, space="PSUM") as pss, \
         tc.tile_pool(name="ps_o", bufs=2, space="PSUM") as pso, \
         tc.tile_pool(name="ps_t", bufs=1, space="PSUM") as pst, \
         tc.tile_pool(name="ps_b", bufs=1, space="PSUM") as psb:
        for hh in range(HPC * attn_scale):
            h = hh % HPC
            kt3 = kt8_t[h][:].rearrange("p (j s) -> p j s", j=2)
            qt3 = qt8_t[h][:].rearrange("p (j s) -> p j s", j=2)
            rsd = rsdp.tile([DH, S], BF16, tag="rsd", name="rsd")
            nc.sync.dma_start(rsd[:], resid_d[h * DH : (h + 1) * DH, :])
            oth = outp_sb.tile([DH, S], BF16, tag="oth", name="oth")
            for qc in range(NQC):
                q0 = qc * QC
                nkt = 4 * qc + 4
                nfull = 4 * qc  # full (non-band) k-tiles
                po = pso.tile([DH, QC], F32, name="po")
                use_pool = opts["rowsum"] == "pool"
                psum = pst.tile([1, QC], F32, name="psum")
                if use_pool:
                    exsum = exsp.tile([DH, QC], F32R, tag="exsum", name="exsum")

                for kt in range(nkt):
                    off = 0 if kt < nfull else (kt - nfull) * DH
                    ps = pss.tile([DH, QC], F32, name="ps")
                    nc.tensor.matmul(
                        ps[:, off:QC],
                        kt3[:, :, kt * DH : (kt + 1) * DH],
                        qt3[:, :, q0 + off : q0 + QC],
                        start=True,
                        stop=True,
                        perf_mode=DR,
                    )
                    ex = exp_p.tile([DH, QC], BF16, tag="ex", name="ex")
                    if fast:
                        nc.scalar.activation(
                            ex[:, off:QC], ps[:, off:QC], EXP, scale=float(SCALE)
                        )
                    else:
                        nc.scalar.activation(
                            ex[:, off:QC], ps[:, off:QC], EXP,
                            bias=kbias[:, kt : kt + 1], scale=float(SCALE),
                        )
                    if kt >= nfull:
                        # causal triangle on the diagonal 128x128 block
                        nc.vector.tensor_mul(
                            ex[:, off : off + DH], ex[:, off : off + DH], tri[:]
                        )
                    if use_pool:
                        # row-sum accumulate on the (otherwise idle) Pool engine
                        if kt == 0:
                            nc.gpsimd.tensor_copy(exsum[:, off:QC], ex[:, off:QC])
                        else:
                            nc.gpsimd.tensor_add(
                                exsum[:, off:QC], exsum[:, off:QC], ex[:, off:QC]
                            )
                    else:
                        nc.tensor.matmul(
                            psum[:, off:QC],
                            ones_cb[:],
                            ex[:, off:QC],
                            start=(kt == 0),
                            stop=(kt == nkt - 1),
                        )
                    nc.tensor.matmul(
                        po[:, off:QC],
                        vfull[:, kt * EH + h * DH : kt * EH + (h + 1) * DH],
                        ex[:, off:QC],
                        start=(kt == 0),
                        stop=(kt == nkt - 1),
                    )
                if use_pool:
                    nc.tensor.matmul(
                        psum[:], ones_c[:], exsum[:], start=True, stop=True
                    )
                rec = tailp.tile([1, QC], F32R, tag="rec", name="rec")
                nc.vector.reciprocal(rec[:], psum[:])
                pbc = psb.tile([DH, QC], F32, name="pbc")
                nc.tensor.matmul(pbc[:], ones_r[:], rec[:], start=True, stop=True)
                bcs = tailp.tile([DH, QC], F32, tag="bcs", name="bcs")
                nc.vector.tensor_copy(bcs[:], pbc[:])
                nc.vector.tensor_mul(bcs[:], po[:], bcs[:])
                nc.vector.tensor_add(
                    oth[:, q0 : q0 + QC], bcs[:], rsd[:, q0 : q0 + QC]
                )
            nc.sync.dma_start(outT_d[h * DH : (h + 1) * DH, :], oth[:])


def _host_prep(queries, keys, Wq, bq, Wk, bk, Wv, bv):
    """Build the 8 per-core input maps (host-side shard + layout prep)."""
    import ml_dtypes

    f8 = ml_dtypes.float8_e4m3
    bf16 = ml_dtypes.bfloat16
    queries = np.ascontiguousarray(queries, dtype=np.float32)
    keys = np.ascontiguousarray(keys, dtype=np.float32)

    def pair_chunks(x):
        # x [S, HID] -> [128p, (sc, f, j, s')] with hid = f*256 + j*128 + p
        a = x.T.reshape(NDR, 2, DH, S)          # [f, j, p, s]
        a = a.transpose(2, 3, 0, 1)             # [p, s, f, j]
        a = a.reshape(DH, NPC, PC, NDR, 2)      # [p, sc, s', f, j]
        a = a.transpose(0, 1, 3, 4, 2)          # [p, sc, f, j, s']
        return np.ascontiguousarray(a.astype(f8)).reshape(DH, -1)

    def pair_weights(w, e0):
        # w [HID(out), HID(in)] -> [128p, (et, f, j, dh)] for out slice
        # e0:e0+EH (et-major so K-proj group et only needs piece et)
        a = w[e0 : e0 + EH, :].T                # [hid, e]
        a = a.reshape(NDR, 2, DH, HPC, DH)      # [f, j, p, et, dh]
        a = a.transpose(2, 3, 0, 1, 4)          # [p, et, f, j, dh]
        return np.ascontiguousarray(a.astype(f8)).reshape(DH, -1)

    qT8_b = [pair_chunks(queries[b]) for b in range(B)]
    kT8_b = [pair_chunks(keys[b]) for b in range(B)]
    residT = [
        np.ascontiguousarray(queries[b].T.astype(bf16)) for b in range(B)
    ]

    Wq = np.asarray(Wq, np.float32)
    Wk = np.asarray(Wk, np.float32)
    Wv = np.asarray(Wv, np.float32)
    bq = np.asarray(bq, np.float32)
    bk = np.asarray(bk, np.float32)
    bv = np.asarray(bv, np.float32)

    ksum = keys.sum(axis=-1)  # [B, S]
    kbias_all = np.where(ksum != 0.0, np.float32(0), NEG_BIAS).astype(np.float32)

    tri = (np.arange(DH)[None, :] >= np.arange(DH)[:, None]).astype(bf16)
    ones_c = np.ones((DH, 1), np.float32)
    ones_cb = np.ones((DH, 1), bf16)
    ones_s = np.ones((1, DH), bf16)
    ones_r = np.ones((1, DH), np.float32)

    in_maps = []
    for c in range(NCORES):
        b, hg = divmod(c, 2)
        e0 = hg * EH
        in_maps.append(
            {
                "qT8": qT8_b[b],
                "kT8": kT8_b[b],
                "wq8": pair_weights(Wq, e0),
                "wk8": pair_weights(Wk, e0),
                "wv8": pair_weights(Wv, e0),
                "bq_d": np.ascontiguousarray(bq[e0 : e0 + EH].reshape(HPC, DH).T),
                "bk_d": np.ascontiguousarray(bk[e0 : e0 + EH].reshape(HPC, DH).T),
                "bv_d": np.ascontiguousarray(
                    bv[e0 : e0 + EH].reshape(1, EH)
                ).astype(bf16),
                "kbias_d": np.ascontiguousarray(kbias_all[b].reshape(NKT, DH).T),
                "tri_d": tri,
                "ones_c_d": ones_c,
                "ones_cb_d": ones_cb,
                "ones_s_d": ones_s,
                "ones_r_d": ones_r,
                "resid_d": residT[b][e0 : e0 + EH, :],
            }
        )
    return in_maps


def _assemble(results):
    """results: list of 8 dicts with outT_d [EH, S] bf16 -> full [B, S, HID]."""
    out = np.empty((B, S, HID), np.float32)
    for c in range(NCORES):
        b, hg = divmod(c, 2)
        out[b, :, hg * EH : (hg + 1) * EH] = results[c]["outT_d"].T.astype(
            np.float32
        )
    return out


def kernel(**inputs):
    from concourse.bass_utils import run_bass_kernel_spmd

    # fast path is valid unless some key row is exactly zero-sum (padding)
    keys = np.asarray(inputs["keys"], np.float32)
    fast = not bool(np.any(keys.sum(axis=-1) == 0.0))
    nc = _build(fast=fast)
    in_maps = _host_prep(**inputs)
    res = run_bass_kernel_spmd(nc, in_maps, core_ids=list(range(NCORES)))
    kernel.last_results = res
    return _assemble(res.results)


def _head_view(tiles, h):
    """[64, 2, S] pair-layout view of head h inside its packed tile."""
    lo = (h % 2) * 64
    v = tiles[h // 2][:].rearrange("p (j s) -> p j s", j=2)
    return v[lo : lo + 64]


def _fused_body(
    nc, tc, phases, scale, fast, opts,
    kT4, qT4, wk8, wv8, wq8,
    kt8_t, qt8_t, vfull, resid_d, outT_d,
    tri, kbias, ones_c, ones_s, ones_r,
    bq_sb, bk_sb, bv_sb,
    F32, F32R, F8, BF16, EXP, IDENT, mybir,
):
    """Chunk-level pipeline: for each s-chunk sc, project K/V/Q(sc) then run
    attention for q-chunk sc over all heads; chunk sc+1's projections overlap
    attention sc on disjoint engines/PSUM banks."""
    import contextlib

    ones_c, ones_cb = ones_c
    DR = mybir.MatmulPerfMode.DoubleRow

    with contextlib.ExitStack() as st:
        wkp = st.enter_context(tc.tile_pool(name="wk", bufs=1))
        wvp = st.enter_context(tc.tile_pool(name="wv", bufs=1))
        wqp = st.enter_context(tc.tile_pool(name="wq", bufs=1))
        kcp = st.enter_context(tc.tile_pool(name="kc", bufs=opts.get("ckb", 2)))
        qcp = st.enter_context(tc.tile_pool(name="qc", bufs=opts.get("ckb", 2)))
        ppp = st.enter_context(
            tc.tile_pool(name="pp", bufs=opts.get("pp", 2), space="PSUM"))
        exp_p = st.enter_context(tc.tile_pool(name="ex", bufs=opts.get("exb", 4)))
        tailp = st.enter_context(tc.tile_pool(name="tail", bufs=opts.get("tlb", 2)))
        rsdp = st.enter_context(tc.tile_pool(name="rsd", bufs=3))
        outp = st.enter_context(tc.tile_pool(name="outs", bufs=3))
        bc_pool = opts.get("bcast", "pool") == "pool"
        nss = opts.get("pss", 2)
        npp = opts.get("pp", 2)
        nso = 8 - npp - nss - 1 - (0 if bc_pool else 1)
        pss = st.enter_context(tc.tile_pool(name="ps_s", bufs=nss, space="PSUM"))
        pso = st.enter_context(tc.tile_pool(name="ps_o", bufs=nso, space="PSUM"))
        pst = st.enter_context(tc.tile_pool(name="ps_t", bufs=1, space="PSUM"))
        psb = (None if bc_pool else
               st.enter_context(tc.tile_pool(name="ps_b", bufs=1, space="PSUM")))

        ETW = NDR * 2 * DH  # per-et weight block (columns)
        wk_t = wkp.tile([DH, NDR * 2 * EH], F8, tag="wk", name="wk_t")
        wv_t = wvp.tile([DH, NDR * 2 * EH], F8, tag="wv", name="wv_t")
        wq_t = wqp.tile([DH, NDR * 2 * EH], F8, tag="wq", name="wq_t")
        # wk in per-et pieces: K-proj group et starts after its piece lands
        nwkp = opts.get("wk_pieces", 1)
        WPC = NDR * 2 * EH // nwkp
        for i in range(nwkp):
            nc.sync.dma_start(
                wk_t[:, i * WPC : (i + 1) * WPC],
                wk8[:, i * WPC : (i + 1) * WPC])
        # wv/wq halves on the Activation hwdge queue so they don't delay
        # the first K chunk behind them on SP
        HW8 = NDR * 2 * EH // 2
        for lo in (0, HW8):
            nc.scalar.dma_start(wv_t[:, lo : lo + HW8], wv8[:, lo : lo + HW8])
        for lo in (0, HW8):
            nc.scalar.dma_start(wq_t[:, lo : lo + HW8], wq8[:, lo : lo + HW8])
        # [p, et, f, j, dh]
        wk4 = wk_t[:].rearrange("p (et f j d) -> p et f j d", et=HPC, f=NDR, j=2)
        wv4 = wv_t[:].rearrange("p (et f j d) -> p et f j d", et=HPC, f=NDR, j=2)
        wq4 = wq_t[:].rearrange("p (et f j d) -> p et f j d", et=HPC, f=NDR, j=2)

        for sc in range(NPC):
            s0 = sc * PC
            # ---- K/V/Q projections for chunk sc ----
            kc = kcp.tile([DH, NDR * 2 * PC], F8, tag="kc", name="kc")
            kc3 = kc[:].rearrange("p (f j s) -> p f j s", f=NDR, j=2)
            nc.sync.dma_start(kc3, kT4[:, sc])
            qch = qcp.tile([DH, NDR * 2 * PC], F8, tag="qch", name="qch")
            qc3 = qch[:].rearrange("p (f j s) -> p f j s", f=NDR, j=2)
            nc.sync.dma_start(qc3, qT4[:, sc])
            for et in range(HPC):
                pk = ppp.tile([DH, PC], F32, tag="pp", name="pk")
                for f in range(NDR):
                    nc.tensor.matmul(
                        pk[:], wk4[:, et, f], kc3[:, f],
                        start=(f == 0), stop=(f == NDR - 1), perf_mode=DR)
                kt3 = _head_view(kt8_t, et)
                for j in range(2):
                    dv = opts["drains"] == "dve" or (
                        opts["drains"] == "mix" and j == 1) or (
                        opts.get("last_dve") and sc == NPC - 1)
                    if dv:
                        nc.vector.tensor_scalar(
                            kt3[:, j, s0 : s0 + PC],
                            pk[j * 64 : (j + 1) * 64, :],
                            bk_sb[j * 64 : (j + 1) * 64, et : et + 1],
                            None, mybir.AluOpType.add)
                    else:
                        nc.scalar.activation(
                            kt3[:, j, s0 : s0 + PC],
                            pk[j * 64 : (j + 1) * 64, :], IDENT,
                            bias=bk_sb[j * 64 : (j + 1) * 64, et : et + 1])
            for sti in range(PC // DH):
                st_g = sc * (PC // DH) + sti
                for ec in range(EH // QC):
                    pv = ppp.tile([DH, QC], F32, tag="pp", name="pv")
                    nhe = QC // DH  # et blocks per e-chunk
                    for f in range(NDR):
                        wv_r = wv4[:, ec * nhe : (ec + 1) * nhe, f].rearrange(
                            "p et j d -> p j et d")
                        nc.tensor.matmul(
                            pv[:],
                            kc3[:, f, :, sti * DH : (sti + 1) * DH],
                            wv_r,
                            start=(f == 0), stop=False, perf_mode=DR)
                    nc.tensor.matmul(
                        pv[:], ones_s[:], bv_sb[:, ec * QC : (ec + 1) * QC],
                        start=False, stop=True)
                    if opts.get("vdrain", "dve") == "pool":
                        nc.gpsimd.tensor_copy(
                            vfull[:, st_g * EH + ec * QC : st_g * EH + (ec + 1) * QC],
                            pv[:])
                    else:
                        nc.vector.tensor_copy(
                            vfull[:, st_g * EH + ec * QC : st_g * EH + (ec + 1) * QC],
                            pv[:])
            for et in range(HPC):
                pq = ppp.tile([DH, PC], F32, tag="pp", name="pq")
                for f in range(NDR):
                    nc.tensor.matmul(
                        pq[:], wq4[:, et, f], qc3[:, f],
                        start=(f == 0), stop=(f == NDR - 1), perf_mode=DR)
                qt3 = _head_view(qt8_t, et)
                for j in range(2):
                    dv = opts["drains"] == "dve" or (
                        opts["drains"] == "mix" and j == 1) or (
                        opts.get("last_dve") and sc == NPC - 1)
                    if dv:
                        nc.vector.tensor_scalar(
                            qt3[:, j, s0 : s0 + PC],
                            pq[j * 64 : (j + 1) * 64, :],
                            bq_sb[j * 64 : (j + 1) * 64, et : et + 1],
                            None, mybir.AluOpType.add)
                    else:
                        nc.scalar.activation(
                            qt3[:, j, s0 : s0 + PC],
                            pq[j * 64 : (j + 1) * 64, :], IDENT,
                            bias=bq_sb[j * 64 : (j + 1) * 64, et : et + 1])

            # pass A of the last q-chunk (k-tiles 0..7): emitted before
            # attention(qc=2) -- its deps (K chunks 0-1, Q chunk 3) are
            # ready, so it fills engines while attention waits on K(2)
            if split3 and sc == 2:
                q0p = (NQC - 1) * QC
                for h in range(HPC):
                    kt3 = _head_view(kt8_t, h)
                    qt3 = _head_view(qt8_t, h)
                    poA = ppp.tile([DH, QC], F32, tag="pp", name="poA")
                    psA = pst.tile([1, QC], F32, name="psum")
                    for kt in range(8):
                        ps = pss.tile([DH, QC], F32, name="ps")
                        nc.tensor.matmul(
                            ps[:],
                            kt3[:, :, kt * DH : (kt + 1) * DH],
                            qt3[:, :, q0p : q0p + QC],
                            start=True, stop=True, perf_mode=DR)
                        ex = exp_p.tile([DH, QC], BF16, tag="ex", name="ex")
                        if fast:
                            nc.scalar.activation(
                                ex[:], ps[:], EXP, scale=float(SCALE))
                        else:
                            nc.scalar.activation(
                                ex[:], ps[:], EXP,
                                bias=kbias[:, kt : kt + 1], scale=float(SCALE))
                        nc.tensor.matmul(
                            psA[:], ones_cb[:], ex[:],
                            start=(kt == 0), stop=(kt == 7))
                        nc.tensor.matmul(
                            poA[:],
                            vfull[:, kt * EH + h * DH : kt * EH + (h + 1) * DH],
                            ex[:],
                            start=(kt == 0), stop=(kt == 7))
                    nc.gpsimd.tensor_copy(
                        poa_all[:, h * QC : (h + 1) * QC], poA[:])
                    nc.gpsimd.tensor_copy(
                        rsA_all[:, h * QC : (h + 1) * QC], psA[:])

            # ---- attention for q-chunk sc, all heads ----
            qc_ = sc
            q0 = qc_ * QC
            nkt = 4 * qc_ + 4
            nfull = 4 * qc_
            use_pool = opts["rowsum"] == "pool"
            kt_lo = 8 if (split3 and qc_ == NQC - 1) else 0
            for h in range(HPC):
                kt3 = _head_view(kt8_t, h)
                qt3 = _head_view(qt8_t, h)
                rsd = rsdp.tile([DH, QC], BF16, tag="rsd", name="rsd")
                nc.gpsimd.dma_start(
                    rsd[:], resid_d[h * DH : (h + 1) * DH, q0 : q0 + QC])
                po = pso.tile([DH, QC], F32, name="po")
                psum = pst.tile([1, QC], F32, name="psum")
                if use_pool:
                    exsum = tailp.tile([DH, QC], F32R, tag="exsum", name="exsum")
                for kt in range(kt_lo, nkt):
                    off = 0 if kt < nfull else (kt - nfull) * DH
                    ps = pss.tile([DH, QC], F32, name="ps")
                    nc.tensor.matmul(
                        ps[:, off:QC],
                        kt3[:, :, kt * DH : (kt + 1) * DH],
                        qt3[:, :, q0 + off : q0 + QC],
                        start=True, stop=True, perf_mode=DR)
                    ex = exp_p.tile([DH, QC], BF16, tag="ex", name="ex")
                    if fast:
                        nc.scalar.activation(
                            ex[:, off:QC], ps[:, off:QC], EXP, scale=float(SCALE))
                    else:
                        nc.scalar.activation(
                            ex[:, off:QC], ps[:, off:QC], EXP,
                            bias=kbias[:, kt : kt + 1], scale=float(SCALE))
                    if kt >= nfull:
                        if opts.get("tri", "dve") == "pool":
                            nc.gpsimd.tensor_mul(
                                ex[:, off : off + DH], ex[:, off : off + DH],
                                tri[:])
                        else:
                            nc.vector.tensor_mul(
                                ex[:, off : off + DH], ex[:, off : off + DH],
                                tri[:])
                    if use_pool:
                        if kt == kt_lo:
                            nc.gpsimd.tensor_copy(exsum[:, off:QC], ex[:, off:QC])
                        else:
                            nc.gpsimd.tensor_add(
                                exsum[:, off:QC], exsum[:, off:QC], ex[:, off:QC])
                    else:
                        nc.tensor.matmul(
                            psum[:, off:QC], ones_cb[:], ex[:, off:QC],
                            start=(kt == kt_lo), stop=(kt == nkt - 1))
                    nc.tensor.matmul(
                        po[:, off:QC],
                        vfull[:, kt * EH + h * DH : kt * EH + (h + 1) * DH],
                        ex[:, off:QC],
                        start=(kt == kt_lo), stop=(kt == nkt - 1))
                if use_pool:
                    nc.tensor.matmul(
                        psum[:], ones_c[:], exsum[:], start=True, stop=True)
                rec = tailp.tile([1, QC], F32R, tag="rec", name="rec")
                if kt_lo:
                    dsum = tailp.tile([1, QC], F32, tag="dsum", name="dsum")
                    nc.vector.tensor_add(
                        dsum[:], psum[:], rsA_all[:, h * QC : (h + 1) * QC])
                    nc.vector.reciprocal(rec[:], dsum[:])
                else:
                    nc.vector.reciprocal(rec[:], psum[:])
                bcs = tailp.tile(
                    [DH, QC], BF16 if opts.get("bcs16") else F32R,
                    tag="bcs", name="bcs")
                if bc_pool:
                    nc.gpsimd.partition_broadcast(bcs[:], rec[:])
                else:
                    pbc = psb.tile([DH, QC], F32, name="pbc")
                    nc.tensor.matmul(
                        pbc[:], ones_r[:], rec[:], start=True, stop=True)
                    nc.vector.tensor_copy(bcs[:], pbc[:])
                if kt_lo:
                    tmp = tailp.tile([DH, QC], F32, tag="tmp", name="tmp")
                    nc.vector.tensor_add(
                        tmp[:], po[:], poa_all[:, h * QC : (h + 1) * QC])
                    nc.vector.tensor_mul(bcs[:], tmp[:], bcs[:])
                else:
                    nc.vector.tensor_mul(bcs[:], po[:], bcs[:])
                oth = outp.tile([DH, QC], BF16, tag="oth", name="oth")
                if opts.get("radd", "dve") == "pool":
                    nc.gpsimd.tensor_add(oth[:], bcs[:], rsd[:])
                else:
                    nc.vector.tensor_add(oth[:], bcs[:], rsd[:])
                nc.gpsimd.dma_start(
                    outT_d[h * DH : (h + 1) * DH, q0 : q0 + QC], oth[:])

